# revision 22
# baseline (speedup 1.0000x reference)
"""Bass/Trainium2 kernel for nn_MemoryGAT (3-layer GATv2 + MLP head), 8 NeuronCores.

Nodes are degree-balanced into 8x98 tiles of 128 (K edge-tiles per node tile,
K~4). Per layer: finalize+projection loop (LN via batched Sqrt over per-tile
Sum(z)/Sum(z^2), gelu, hs/ht/skip matmuls, chunked AllGather of hs), then the
edge loop (indirect gathers of hs[src], one-hot selection matmuls, softmax
without max-subtraction, scatter-add matmul) fused with LN-stat accumulation.
ht/z stay in SBUF; activation-table churn is eliminated by keeping the ACT
engine on one function per phase (Prelu/Copy/Square live in every table).
"""

import sys
import types

sys.path.insert(0, "/opt/trn_rl_repo")

import ml_dtypes
import numpy as np
import orjson

# ---------------------------------------------------------------- shims

_counter = [0]


def _legalize_module(m, maxw=1):
    """This walrus build accepts only ONE sync-wait per instruction; hoist
    overflow waits onto NoOps inserted just before, on the same engine."""
    for f in m.get("functions", []):
        for b in f.get("blocks", []):
            insts = b.get("instructions")
            if not insts:
                continue
            out = []
            for inst in insts:
                si = inst.get("sync_info")
                waits = (si or {}).get("on_wait") or []
                if si is not None and len(waits) > maxw:
                    keep = waits[-maxw:]
                    extra = waits[: len(waits) - maxw]
                    for j in range(0, len(extra), maxw):
                        _counter[0] += 1
                        out.append(
                            {
                                "name": f"ant-wsplit-{_counter[0]}",
                                "opcode": "NoOp",
                                "engine": inst.get("engine"),
                                "ins": [],
                                "outs": [],
                                "sync_info": {
                                    "on_wait": extra[j : j + maxw],
                                    "on_update": [],
                                },
                            }
                        )
                    si["on_wait"] = keep
                out.append(inst)
            b["instructions"] = out
    return m


def _install_shims():
    import antenv

    if "antenv.axon_hooks" not in sys.modules:
        try:
            from trn_agent_boot.trn_boot import _ntff_profile_via_ctypes

            hooks = types.ModuleType("antenv.axon_hooks")
            hook = _ntff_profile_via_ctypes("/opt/axon/libaxon_pjrt.so")
            hooks.get_axon_ntff_profile_hook = lambda: hook
            hooks.set_axon_ntff_profile_hook = lambda h: None
            sys.modules["antenv.axon_hooks"] = hooks
            antenv.axon_hooks = hooks
        except Exception:
            pass

    import concourse.bass as bass
    from concourse import bass_utils

    bass_utils.upload_artifacts = lambda tmpdir: tmpdir

    if not getattr(bass.Bass, "_waitfix_installed", False):
        base = bass.Bass.to_json_bytes

        def patched(self):
            return orjson.dumps(_legalize_module(orjson.loads(base(self))))

        bass.Bass.to_json_bytes = patched
        bass.Bass._waitfix_installed = True


_install_shims()

import concourse.bass as bass
import concourse.tile as tile
from concourse import mybir
from concourse.bass_utils import run_bass_kernel_spmd

F32 = mybir.dt.float32
BF = mybir.dt.bfloat16
AF = mybir.ActivationFunctionType
ALU = mybir.AluOpType

# ---------------------------------------------------------------- sizes
N = 100_000
E = 400_000
FN = 267
DC = 256
H, D = 4, 64
HD = 256
ED = 11
NCORES = 8
P = 128
NT = 98
NPAD = NT * P  # 12544
NFULL = NCORES * NPAD
# AllGather chunk boundaries (in node tiles) and hs_full region bases
CHT = [0, 48, 76, 98]  # tile boundaries of the AG chunks
NCH = len(CHT) - 1
CHROWS = [(CHT[i + 1] - CHT[i]) * P for i in range(NCH)]  # rows/core per chunk
CHBASE = [0]
for i in range(NCH - 1):
    CHBASE.append(CHBASE[-1] + NCORES * CHROWS[i])

TRACE = False
LAST_RESULT = {}


def _row_of(lt, slot, core):
    """hs_full row for node at (core, local tile lt, slot)."""
    for c in range(NCH):
        if lt < CHT[c + 1]:
            return CHBASE[c] + core * CHROWS[c] + (lt - CHT[c]) * P + slot
    raise AssertionError


# ---------------------------------------------------------------- builder
def build_nc(K, bh2_val):
    NTK = NT * K
    ES = NTK * P

    nc = bass.Bass()
    dp = nc.declare_dram_parameter

    x_T = dp("x_T", [384, NPAD], BF, isOutput=False)
    src_c = dp("src_c", [P, NTK], mybir.dt.int32, isOutput=False)
    tgt_c = dp("tgt_c", [P, NTK], BF, isOutput=False)
    ea_T = dp("ea_T", [16, ES], BF, isOutput=False)
    wp1 = dp("wp1", [384, 64], BF, isOutput=False)
    gin_rep = dp("gin_rep", [P, 64], F32, isOutput=False)
    bin_rep = dp("bin_rep", [P, 64], F32, isOutput=False)
    iota2d = dp("iota2d", [P, P], BF, isOutput=False)
    ident = dp("ident", [P, P], BF, isOutput=False)
    wh1 = dp("wh1", [64, 32], F32, isOutput=False)
    bh1_rep = dp("bh1_rep", [P, 32], F32, isOutput=False)
    wh2_rep = dp("wh2_rep", [P, 32], F32, isOutput=False)

    LW = []
    for l, ind in ((0, 65), (1, 256), (2, 256)):
        d = {"ind": ind, "outd": 64 if l == 2 else 256}
        d["wswt"] = dp(f"wswt{l}", [ind, 512], BF, isOutput=False)
        d["we"] = dp(f"we{l}", [16, 256], BF, isOutput=False)
        d["a_rep"] = dp(f"a_rep{l}", [P, 256], BF, isOutput=False)
        if l != 1:
            d["skw"] = dp(f"skw{l}", [ind, d["outd"]], BF, isOutput=False)
            d["skb_rep"] = dp(f"skb_rep{l}", [P, d["outd"]], F32, isOutput=False)
        d["gn_rep"] = dp(f"gn_rep{l}", [P, d["outd"]], F32, isOutput=False)
        d["bn_rep"] = dp(f"bn_rep{l}", [P, d["outd"]], F32, isOutput=False)
        LW.append(d)

    out = dp("out", [P, NT], F32, isOutput=True)

    hs_shard = [nc.dram_tensor(f"hs_shard{l}", [NPAD, 256], BF) for l in range(3)]
    hs_full = [
        nc.dram_tensor(f"hs_full{l}", [NFULL, 256], BF, addr_space="Shared")
        for l in range(3)
    ]
    res0_dram = nc.dram_tensor("res0_dram", [NPAD, 256], BF)
    h1_dram = nc.dram_tensor("h1_dram", [NPAD, 256], BF)
    z_dram = nc.dram_tensor("z_dram", [NPAD, 256], BF)

    with tile.TileContext(nc) as tc:
        with (
            tc.tile_pool(name="const", bufs=1) as cpool,
            tc.tile_pool(name="work", bufs=2) as wpool,
            tc.tile_pool(name="small", bufs=2) as spool,
            tc.tile_pool(name="persist", bufs=1) as ppool,
            tc.tile_pool(name="psA", bufs=2, space="PSUM") as psA,
            tc.tile_pool(name="psB", bufs=2, space="PSUM") as psB,
            tc.tile_pool(name="psC", bufs=2, space="PSUM") as psC,
            tc.tile_pool(name="psD", bufs=2, space="PSUM") as psD,
        ):
            for v in {1e-5, 1e-8, float(bh2_val)}:
                ct = cpool.tile([P, 1], F32, tag=f"k{v}", name=f"k{_counter[0]}")
                _counter[0] += 1
                nc.vector.memset(ct[:], v)
                nc.const_aps.aps[(F32, float(v))] = ct[:]

            _cn = [0]

            def c_load(ap, shape, dt=F32):
                _cn[0] += 1
                t = cpool.tile(shape, dt, tag=f"c{_cn[0]}", name=f"c{_cn[0]}")
                nc.sync.dma_start(out=t[:], in_=ap[:])
                return t

            def c_load_chunks(ap, kk, ck, n, dt=F32):
                _cn[0] += 1
                t = cpool.tile([kk, ck * n], dt, tag=f"c{_cn[0]}", name=f"c{_cn[0]}")
                for c in range(ck):
                    nc.sync.dma_start(
                        out=t[:, c * n : (c + 1) * n],
                        in_=ap[c * kk : (c + 1) * kk, :],
                    )
                return t

            iota_sb = c_load(iota2d, [P, P], BF)
            idb_sb = c_load(ident, [P, P], BF)
            iotaK_sb = cpool.tile([P, K * P], BF, tag="iotaK", name="iotaK")
            for k in range(K):
                nc.vector.tensor_copy(iotaK_sb[:, k * P : (k + 1) * P], iota_sb[:])
            wp1_sb = c_load_chunks(wp1, P, 3, 64, BF)
            gin_sb = c_load(gin_rep, [P, 64])
            bin_sb = c_load(bin_rep, [P, 64])
            wh1_sb = c_load(wh1, [64, 32])
            bh1_sb = c_load(bh1_rep, [P, 32])
            wh2_sb = c_load(wh2_rep, [P, 32])
            lws = []
            for l, d in enumerate(LW):
                s = {}
                ck = max(d["ind"] // P, 1)
                kk = min(d["ind"], P)
                s["wswt"] = c_load_chunks(d["wswt"], kk, ck, 512, BF)
                s["we"] = c_load(d["we"], [16, 256], BF)
                s["a_rep"] = c_load(d["a_rep"], [P, 256], BF)
                if "skw" in d:
                    s["skw"] = c_load_chunks(d["skw"], kk, ck, d["outd"], BF)
                    s["skb"] = c_load(d["skb_rep"], [P, d["outd"]])
                s["gn"] = c_load(d["gn_rep"], [P, d["outd"]])
                s["bn"] = c_load(d["bn_rep"], [P, d["outd"]])
                s["ck"], s["kk"] = ck, kk
                lws.append(s)

            srcs = ppool.tile([P, NTK], mybir.dt.int32)
            nc.sync.dma_start(out=srcs[:], in_=src_c[:])
            tgts = ppool.tile([P, NTK], BF)
            nc.sync.dma_start(out=tgts[:], in_=tgt_c[:])

            h0T = ppool.tile([65, NPAD], BF)
            nc.vector.memset(h0T[64:65, :], 1.0)
            ht_all = ppool.tile([P, NT * 256], BF)
            res2_all = ppool.tile([P, NT * 64], BF)
            zview = z_dram[:].rearrange("(t p) c -> p t c", p=P)
            scores = ppool.tile([P, NT], F32)

            # one shared LN-stat set; stages are strictly phased so WAR
            # deps keep this safe
            _st = {}
            for nm in ("s1", "s2", "m", "va", "rstd"):
                _st[nm] = ppool.tile([P, NT], F32, tag=f"st{nm}", name=f"st{nm}")
            stats = [_st] * 4

            def sqrt_batch(i, dim, c0=0, c1=NT):
                st = stats[i]
                cs = slice(c0, c1)
                nc.vector.tensor_scalar_mul(st["m"][:, cs], st["s1"][:, cs], 1.0 / dim)
                nc.vector.tensor_scalar_mul(st["va"][:, cs], st["s2"][:, cs], 1.0 / dim)
                nm2 = spool.tile([P, NT], F32, tag="nm2")
                nc.vector.scalar_tensor_tensor(
                    nm2[:, cs], st["m"][:, cs], -1.0, st["m"][:, cs],
                    op0=ALU.mult, op1=ALU.mult,
                )
                nc.vector.tensor_add(st["va"][:, cs], st["va"][:, cs], nm2[:, cs])
                sd = spool.tile([P, NT], F32, tag="sd")
                nc.scalar.activation(sd[:, cs], st["va"][:, cs], AF.Sqrt, bias=1e-5)
                nc.vector.reciprocal(st["rstd"][:, cs], sd[:, cs])
                nc.vector.scalar_tensor_tensor(
                    st["va"][:, cs], st["m"][:, cs], -1.0, st["rstd"][:, cs],
                    op0=ALU.mult, op1=ALU.mult,
                )

            def ag_chunk(l, c):
                nc.gpsimd.collective_compute(
                    "AllGather",
                    ALU.bypass,
                    ins=[hs_shard[l][CHT[c] * P : CHT[c + 1] * P, :]],
                    outs=[
                        hs_full[l][CHBASE[c] : CHBASE[c] + NCORES * CHROWS[c], :]
                    ],
                    replica_groups=[list(range(NCORES))],
                )

            # ---------------- phase 0: u = x@Wp (+ctx/bias via ones-row),
            # z0 = gelu(u), accumulate LN stats -------------------------
            with nc.named_scope("p0"):
                for t in range(NT):
                    if t % 2 == 0:
                        xt = wpool.tile([P, 3, 256], BF, tag="xt", bufs=2)
                        for c in range(3):
                            nc.sync.dma_start(
                                out=xt[:, c, :],
                                in_=x_T[c * P : (c + 1) * P, t * P : (t + 2) * P],
                            )
                    xoff = (t % 2) * P
                    h0p = psD.tile([P, 64], F32, tag="proj")
                    for c in range(3):
                        nc.tensor.matmul(
                            out=h0p[:],
                            lhsT=xt[:, c, xoff : xoff + P],
                            rhs=wp1_sb[:, c * 64 : (c + 1) * 64],
                            start=(c == 0),
                            stop=(c == 2),
                        )
                    if t % 4 == 0:
                        zstag0 = wpool.tile([P, 4, 64], BF, tag="zstag0", bufs=2)
                    zslot = zstag0[:, t % 4, :]
                    nc.scalar.activation(
                        zslot, h0p[:], AF.Gelu, accum_out=stats[0]["s1"][:, t : t + 1]
                    )
                    junk = wpool.tile([P, 64], BF, tag="junk0", bufs=2)
                    nc.vector.scalar_tensor_tensor(
                        junk[:],
                        zslot,
                        1.0,
                        zslot,
                        op0=ALU.mult,
                        op1=ALU.mult,
                        accum_out=stats[0]["s2"][:, t : t + 1],
                    )
                    if t % 4 == 3 or t == NT - 1:
                        t0 = t - (t % 4)
                        nc.sync.dma_start(
                            out=zview[:, t0 : t + 1, 0:64],
                            in_=zstag0[:, : t - t0 + 1, :],
                        )
                sqrt_batch(0, 64)

            # ---------------- F2A(l): finalize h_l, project, AG ------------
            def f2a(l, t0=0, t1=NT):
                """l in 0..3; finalize h_l from z stats[l], then projections
                for layer l (l<3) or the score head (l==3)."""
                st = stats[l]
                ind = 64 if l == 0 else (256 if l < 3 else 64)
                s = lws[l] if l < 3 else None
                for t in range(t0, t1):
                    if t % 4 == 0:
                        nbt = min(4, NT - t)
                        z_sb = wpool.tile([P, 4, 256], BF, tag="z_sb", bufs=3)
                        nc.sync.dma_start(
                            out=z_sb[:, :nbt, :ind],
                            in_=zview[:, t : t + nbt, :ind],
                        )
                    zslot = z_sb[:, t % 4, :ind]
                    hn = wpool.tile([P, 256], BF, tag="hn", bufs=3, name="hn")[:, :ind]
                    if l == 0:
                        # g_in/b_in folded into wswt0/skw0 via the ones-row;
                        # m holds -mean*rstd after sqrt_batch
                        nc.scalar.activation(
                            hn, zslot, AF.Identity,
                            bias=st["va"][:, t : t + 1],
                            scale=st["rstd"][:, t : t + 1],
                        )
                    else:
                        t1 = wpool.tile([P, 256], F32, tag="t1", bufs=3, name="t1")[:, :ind]
                        g_sb = lws[l - 1]["gn"]
                        b_sb = lws[l - 1]["bn"]
                        nc.vector.scalar_tensor_tensor(
                            t1, zslot, st["m"][:, t : t + 1], g_sb[:, :ind],
                            op0=ALU.subtract, op1=ALU.mult,
                        )
                        u = wpool.tile([P, 256], F32, tag="u", bufs=3, name="u")[:, :ind]
                        nc.vector.scalar_tensor_tensor(
                            u, t1, st["rstd"][:, t : t + 1], b_sb[:, :ind],
                            op0=ALU.mult, op1=ALU.add,
                        )
                        nc.scalar.activation(hn, u, AF.Gelu)
                    if l == 3:
                        # score head
                        trp = psB.tile([P, P], BF, tag="tr")
                        nc.tensor.transpose(out=trp[:64, :], in_=hn, identity=idb_sb[:])
                        h3T = wpool.tile([64, P], F32, tag="h3T", bufs=2)
                        nc.scalar.copy(h3T[:], trp[:64, :])
                        sp1 = psC.tile([P, 64], F32, tag="agg", name="sp1")[:, :32]
                        nc.tensor.matmul(
                            out=sp1, lhsT=h3T[:], rhs=wh1_sb[:], start=True, stop=True
                        )
                        u1 = wpool.tile([P, 32], F32, tag="u1", bufs=2)
                        nc.vector.tensor_add(u1[:], sp1, bh1_sb[:])
                        g1 = wpool.tile([P, 32], F32, tag="g1", bufs=2)
                        nc.scalar.activation(g1[:], u1[:], AF.Gelu)
                        j32 = wpool.tile([P, 32], BF, tag="j32", bufs=2)
                        nc.vector.scalar_tensor_tensor(
                            j32[:], g1[:], 1.0, wh2_sb[:],
                            op0=ALU.mult, op1=ALU.mult,
                            accum_out=scores[:, t : t + 1],
                        )
                        continue
                    # transpose hn for projections
                    ck = s["ck"]
                    if l == 0:
                        trp = psB.tile([P, P], BF, tag="tr")
                        nc.tensor.transpose(out=trp[:64, :], in_=hn, identity=idb_sb[:])
                        nc.scalar.copy(h0T[0:64, t * P : (t + 1) * P], trp[:64, :])
                        lhs = [h0T[:, t * P : (t + 1) * P]]  # 65 rows: 64 data + ones
                    else:
                        hnT = wpool.tile([P, 2, P], BF, tag="hnT", bufs=3)
                        for c in range(2):
                            trp = psB.tile([P, P], BF, tag="tr")
                            nc.tensor.transpose(
                                out=trp[:], in_=hn[:, c * P : (c + 1) * P],
                                identity=idb_sb[:],
                            )
                            nc.scalar.copy(hnT[:, c, :], trp[:])
                        lhs = [hnT[:, c, :] for c in range(2)]
                    hsht = psD.tile([P, 512], F32, tag="proj")
                    for c in range(ck):
                        nc.tensor.matmul(
                            out=hsht[:],
                            lhsT=lhs[c],
                            rhs=s["wswt"][:, c * 512 : (c + 1) * 512],
                            start=(c == 0),
                            stop=(c == ck - 1),
                        )
                    if t % 4 == 0:
                        stag_hs = wpool.tile([P, 4, 256], BF, tag="stag_hs", bufs=2)
                    nc.scalar.copy(stag_hs[:, t % 4, :], hsht[:, 0:256])
                    nc.scalar.copy(
                        ht_all[:, t * 256 : (t + 1) * 256], hsht[:, 256:512]
                    )
                    if l != 1:
                        outd = s and LW[l]["outd"]
                        rp = psC.tile([P, 512], F32, tag="agg", name="rp")[:, :outd]
                        for c in range(ck):
                            nc.tensor.matmul(
                                out=rp,
                                lhsT=lhs[c],
                                rhs=s["skw"][:, c * outd : (c + 1) * outd],
                                start=(c == 0),
                                stop=(c == ck - 1),
                            )
                        if l == 0:
                            if t % 4 == 0:
                                stag_res = wpool.tile(
                                    [P, 4, 256], BF, tag="stag_res", bufs=2
                                )
                            nc.scalar.copy(stag_res[:, t % 4, :], rp)
                        else:
                            nc.vector.scalar_tensor_tensor(
                                res2_all[:, t * 64 : (t + 1) * 64], rp, 1.0,
                                s["skb"][:], op0=ALU.mult, op1=ALU.add,
                            )
                    if l == 1:
                        nc.sync.dma_start(
                            out=h1_dram[:].rearrange("(t p) c -> p t c", p=P)[
                                :, t, :
                            ],
                            in_=hn,
                        )
                    # batched stores + AG chunks
                    if t % 4 == 3 or t == NT - 1:
                        t0 = t - (t % 4)
                        nbt = t - t0 + 1
                        nc.sync.dma_start(
                            out=hs_shard[l][:].rearrange("(t p) c -> p t c", p=P)[
                                :, t0 : t0 + nbt, :
                            ],
                            in_=stag_hs[:, :nbt, :],
                        )
                        if l == 0:
                            nc.sync.dma_start(
                                out=res0_dram[:].rearrange("(t p) c -> p t c", p=P)[
                                    :, t0 : t0 + nbt, :
                                ],
                                in_=stag_res[:, :nbt, :],
                            )
                    for c in range(NCH):
                        if t == CHT[c + 1] - 1:
                            ag_chunk(l, c)

            # ---------------- edge + F1 loop --------------------------------
            def edge_f1(l, t0=0, t1=NT):
                s = lws[l]
                outd = LW[l]["outd"]
                st = stats[l + 1]
                for t in range(t0, t1):
                    if l < 2 and t % 4 == 0:
                        nbt = min(4, NT - t)
                        res_sb = wpool.tile([P, 4, 256], BF, tag="res_sb", bufs=2)
                        rdram = res0_dram if l == 0 else h1_dram
                        nc.sync.dma_start(
                            out=res_sb[:, :nbt, :],
                            in_=rdram[:].rearrange("(t p) c -> p t c", p=P)[
                                :, t : t + nbt, :
                            ],
                        )
                    if t % 2 == 0:
                        ea_sb = wpool.tile([16, 2 * K * P], BF, tag="ea_sb", bufs=2)
                        nc.sync.dma_start(
                            out=ea_sb[:],
                            in_=ea_T[:, t * K * P : (t + 2) * K * P],
                        )
                    eoff = (t % 2) * K * P
                    hs_g = wpool.tile([P, K * 256], BF, tag="hs_g", bufs=3)
                    for k in range(K):
                        nc.gpsimd.indirect_dma_start(
                            out=hs_g[:, k * 256 : (k + 1) * 256],
                            out_offset=None,
                            in_=hs_full[l][:],
                            in_offset=bass.IndirectOffsetOnAxis(
                                ap=srcs[:, t * K + k : t * K + k + 1], axis=0
                            ),
                        )
                    S_all = wpool.tile([P, K * P], BF, tag="S_all", bufs=3)
                    alph = spool.tile([P, K * 4], F32, tag="alph", bufs=2)
                    nc.vector.tensor_tensor(
                        out=S_all[:].rearrange("p (k c) -> p k c", k=K),
                        in0=iotaK_sb[:].rearrange("p (k c) -> p k c", k=K),
                        in1=tgts[:, t * K : (t + 1) * K].to_broadcast((P, K, P)),
                        op=ALU.is_equal,
                    )
                    lr_all = wpool.tile([P, K * 256], BF, tag="lr_all", bufs=3)
                    for k in range(K):
                        Sk = S_all[:, k * P : (k + 1) * P]
                        trp = psB.tile([P, P], BF, tag="tr")
                        nc.tensor.transpose(out=trp[:], in_=Sk, identity=idb_sb[:])
                        ST = wpool.tile([P, P], BF, tag="ST", bufs=3)
                        nc.scalar.copy(ST[:], trp[:])
                        msg = psA.tile([P, 256], F32, tag="msg")
                        nc.tensor.matmul(
                            out=msg[:],
                            lhsT=ea_sb[:, eoff + k * P : eoff + (k + 1) * P],
                            rhs=s["we"][:],
                            start=True,
                            stop=False,
                        )
                        nc.tensor.matmul(
                            out=msg[:], lhsT=ST[:],
                            rhs=ht_all[:, t * 256 : (t + 1) * 256],
                            start=False, stop=False,
                        )
                        nc.tensor.matmul(
                            out=msg[:], lhsT=idb_sb[:],
                            rhs=hs_g[:, k * 256 : (k + 1) * 256],
                            start=False, stop=True,
                        )
                        nc.scalar.activation(
                            lr_all[:, k * 256 : (k + 1) * 256], msg[:],
                            AF.Prelu, alpha=0.2,
                        )
                    scr_all = wpool.tile([P, K * 256], BF, tag="scr_all", bufs=2)
                    nc.vector.tensor_tensor(
                        out=scr_all[:].rearrange("p (k c) -> p k c", k=K),
                        in0=lr_all[:].rearrange("p (k c) -> p k c", k=K),
                        in1=s["a_rep"][:]
                        .rearrange("p (o c) -> p o c", o=1)
                        .to_broadcast((P, K, 256)),
                        op=ALU.mult,
                    )
                    nc.vector.tensor_reduce(
                        out=alph[:],
                        in_=scr_all[:].rearrange("p (g d) -> p g d", d=64),
                        axis=mybir.AxisListType.X,
                        op=ALU.add,
                    )
                    expa = spool.tile([P, K * 4], F32, tag="expa", bufs=2)
                    nc.scalar.activation(expa[:], alph[:], AF.Exp)
                    agg = psC.tile([P, 260], F32, tag="agg")
                    w_aug = wpool.tile([P, K * 260], BF, tag="w_aug", bufs=3)
                    w4 = w_aug[:].rearrange("p (k h c) -> p k h c", k=K, c=65)
                    nc.vector.tensor_tensor(
                        out=w4[:, :, :, 0:64],
                        in0=hs_g[:].rearrange("p (k h d) -> p k h d", k=K, d=64),
                        in1=expa[:]
                        .rearrange("p (k h) -> p k h", k=K)
                        .to_broadcast((P, K, 4, 64)),
                        op=ALU.mult,
                    )
                    nc.vector.tensor_copy(
                        w4[:, :, :, 64],
                        expa[:].rearrange("p (k h) -> p k h", k=K),
                    )
                    for k in range(K):
                        nc.tensor.matmul(
                            out=agg[:],
                            lhsT=S_all[:, k * P : (k + 1) * P],
                            rhs=w_aug[:, k * 260 : (k + 1) * 260],
                            start=(k == 0),
                            stop=(k == K - 1),
                        )
                    aggv = agg[:].rearrange("p (h c) -> p h c", c=65)
                    den = spool.tile([P, 4], F32, tag="den", bufs=2)
                    nc.vector.tensor_scalar(
                        den[:], aggv[:, :, 64], 1e-8, None, op0=ALU.add
                    )
                    rden = spool.tile([P, 4], F32, tag="rden", bufs=2)
                    nc.vector.reciprocal(rden[:], den[:])
                    gat = wpool.tile([P, 256], F32, tag="gat", bufs=2)
                    nc.vector.scalar_tensor_tensor(
                        gat[:].rearrange("p (h d) -> p h d", h=4),
                        aggv[:, :, 0:64],
                        0.25 if l == 2 else 1.0,
                        rden[:].to_broadcast((P, 4, 64)),
                        op0=ALU.mult,
                        op1=ALU.mult,
                    )
                    if l == 2:
                        g64 = wpool.tile([P, 64], F32, tag="g64", bufs=2)
                        nc.vector.tensor_reduce(
                            out=g64[:],
                            in_=gat[:].rearrange("p (h d) -> p d h", h=4),
                            axis=mybir.AxisListType.X,
                            op=ALU.add,
                        )
                        zin = g64[:]
                        res_ap = res2_all[:, t * 64 : (t + 1) * 64]
                    else:
                        zin = gat[:]
                        res_ap = res_sb[:, t % 4, :]
                    if t % 4 == 0:
                        zstag = wpool.tile([P, 4, 256], BF, tag="zstag", bufs=2)
                    zslot = zstag[:, t % 4, :outd]
                    nc.vector.scalar_tensor_tensor(
                        zslot, zin, 1.0, res_ap,
                        op0=ALU.mult, op1=ALU.add,
                        accum_out=st["s1"][:, t : t + 1],
                    )
                    junk = wpool.tile([P, 256], BF, tag="junk", bufs=2, name="junk")[:, :outd]
                    nc.scalar.activation(
                        junk, zslot, AF.Square,
                        accum_out=st["s2"][:, t : t + 1],
                    )
                    if t % 4 == 3 or t == NT - 1:
                        t0 = t - (t % 4)
                        nc.sync.dma_start(
                            out=zview[:, t0 : t + 1, :outd],
                            in_=zstag[:, : t - t0 + 1, :outd],
                        )

            with nc.named_scope("f2a0"):
                f2a(0)
            for l in range(3):
                with nc.named_scope(f"edge{l}"):
                    edge_f1(l, 0, 48)
                    sqrt_batch(l + 1, LW[l]["outd"], 0, 48)
                    # interleave layer-l edge tail with layer-(l+1) finalize
                    fb = list(range(0, 48, 8))
                    for i, e0 in enumerate(range(48, NT, 8)):
                        edge_f1(l, e0, min(e0 + 8, NT))
                        if i < len(fb):
                            f2a(l + 1, fb[i], fb[i] + 8)
                    sqrt_batch(l + 1, LW[l]["outd"], 48, NT)
                with nc.named_scope(f"f2a{l + 1}"):
                    f2a(l + 1, 48, NT)

            sig = ppool.tile([P, NT], F32)
            nc.scalar.activation(sig[:], scores[:], AF.Sigmoid, bias=bh2_val)
            nc.sync.dma_start(out=out[:], in_=sig[:])
    return nc


# ---------------------------------------------------------------- host prep
def _balance_nodes(tgt):
    """Degree-balanced assignment of nodes to NCORES*NT tiles of <=128 slots.
    Returns (gtile[node], slot[node], K)."""
    import heapq

    NTILES = NCORES * NT
    deg = np.bincount(tgt, minlength=N)
    order = np.argsort(-deg, kind="stable")
    gtile = np.empty(N, np.int32)
    slot = np.empty(N, np.int32)
    count = np.zeros(NTILES, np.int32)
    load = np.zeros(NTILES, np.int64)
    heap = [(0, t) for t in range(NTILES)]
    heapq.heapify(heap)
    for node in order:
        while True:
            ld, t = heapq.heappop(heap)
            if count[t] < P and ld == load[t]:
                break
        gtile[node] = t
        slot[node] = count[t]
        count[t] += 1
        load[t] += deg[node]
        if count[t] < P:
            heapq.heappush(heap, (int(load[t]), t))
    K = int(np.ceil(load.max() / P))
    return gtile, slot, K


def _prep(inputs):
    ei = np.asarray(inputs["edge_index"]).astype(np.int64)
    src, tgt = ei[0], ei[1]
    ea = np.asarray(inputs["edge_attr"], np.float32)

    gtile, slot, K = _balance_nodes(tgt)
    core_of = gtile // NT
    lt_of = gtile % NT

    # hs_full row id per node (chunk-major layout)
    lt = lt_of.astype(np.int64)
    chunk = np.searchsorted(np.array(CHT[1:-1]), lt, side="right")
    chrows = np.array(CHROWS)[chunk]
    chbase = np.array(CHBASE)[chunk]
    chtile0 = np.array(CHT[:-1])[chunk]
    row_id = chbase + core_of * chrows + (lt - chtile0) * P + slot

    NTK = NT * K
    ES = NTK * P

    e_core = core_of[tgt]
    e_lt = lt_of[tgt]
    e_p = slot[tgt]  # target's slot within its tile
    order = np.lexsort((e_lt, e_core))
    src_s = src[order]
    ea_s = ea[order]
    e_core_s, e_lt_s, e_p_s = e_core[order], e_lt[order], e_p[order]

    grp = e_core_s * NT + e_lt_s
    idx_in_grp = np.zeros(len(grp), np.int64)
    _, first_pos, cnt = np.unique(grp, return_index=True, return_counts=True)
    for fp, c in zip(first_pos, cnt):
        idx_in_grp[fp : fp + c] = np.arange(c)
    assert cnt.max() <= K * P, (cnt.max(), K)

    src_cols = np.zeros((NCORES, P, NTK), np.int32)
    tgt_cols = np.full((NCORES, P, NTK), -1.0, np.float32)
    ea_T = np.zeros((NCORES, 16, ES), np.float32)
    eslot = e_lt_s * (K * P) + idx_in_grp
    col = eslot // P
    row = eslot % P
    src_cols[e_core_s, row, col] = row_id[src_s].astype(np.int32)
    tgt_cols[e_core_s, row, col] = e_p_s.astype(np.float32)
    ea_T[e_core_s[:, None], np.arange(ED)[None, :], eslot[:, None]] = ea_s

    x = np.asarray(inputs["x"], np.float32)
    x_T = np.zeros((NCORES, 384, NPAD), np.float32)  # cast to bf16 per-core below
    pos = lt * P + slot  # position within core [0, NPAD)
    x_T[core_of, :FN, pos] = x
    x_T[core_of, FN, pos] = 1.0  # ones-row carries ctx@Wp+bp via wp1

    rep = lambda v: np.broadcast_to(
        np.asarray(v, np.float32)[None, :], (P, len(np.asarray(v)))
    ).copy()
    bf = lambda a: np.asarray(a).astype(ml_dtypes.bfloat16)

    Wp = np.asarray(inputs["Wp"], np.float32)
    cb = (
        np.asarray(inputs["context_vector"], np.float32) @ Wp[FN:]
        + np.asarray(inputs["bp"], np.float32)
    )
    wp1 = np.zeros((384, 64), np.float32)
    wp1[:FN] = Wp[:FN]
    wp1[FN] = cb
    wp1 = wp1.astype(ml_dtypes.bfloat16)

    common = {
        "wp1": wp1,
        "gin_rep": rep(inputs["g_in"]),
        "bin_rep": rep(inputs["b_in"]),
        "iota2d": np.broadcast_to(
            np.arange(P, dtype=np.float32)[None, :], (P, P)
        ).astype(ml_dtypes.bfloat16),
        "ident": np.eye(P, dtype=np.float32).astype(ml_dtypes.bfloat16),
        "wh1": np.asarray(inputs["Wh1"], np.float32),
        "bh1_rep": rep(inputs["bh1"]),
        "wh2_rep": rep(np.asarray(inputs["Wh2"], np.float32)[:, 0]),
    }
    g_in = np.asarray(inputs["g_in"], np.float32)
    b_in = np.asarray(inputs["b_in"], np.float32)
    for l in range(3):
        sfx = str(l)
        ws = np.asarray(inputs["Ws" + sfx], np.float32)
        wt = np.asarray(inputs["Wt" + sfx], np.float32)
        wswt = np.concatenate([ws, wt], axis=1)
        if l == 0:
            wswt = np.concatenate(
                [g_in[:, None] * wswt, (b_in @ wswt)[None, :]], axis=0
            )
        common[f"wswt{l}"] = bf(wswt)
        we = np.zeros((16, 256), np.float32)
        we[:ED] = np.asarray(inputs["We" + sfx], np.float32)
        common[f"we{l}"] = bf(we)
        common[f"a_rep{l}"] = bf(
            rep(np.asarray(inputs["A" + sfx], np.float32).reshape(-1))
        )
        if l != 1:
            skw = np.asarray(inputs[f"Sk{l}W"], np.float32)
            if l == 0:
                skw = np.concatenate(
                    [
                        g_in[:, None] * skw,
                        (b_in @ skw + np.asarray(inputs["Sk0b"], np.float32))[
                            None, :
                        ],
                    ],
                    axis=0,
                )
            common[f"skw{l}"] = bf(skw)
            common[f"skb_rep{l}"] = rep(inputs[f"Sk{l}b"])
        common[f"gn_rep{l}"] = rep(inputs["gn" + sfx])
        common[f"bn_rep{l}"] = rep(inputs["bn" + sfx])

    in_maps = []
    for c in range(NCORES):
        m = dict(common)
        m["x_T"] = x_T[c].astype(ml_dtypes.bfloat16)
        m["src_c"] = src_cols[c]
        m["tgt_c"] = tgt_cols[c].astype(ml_dtypes.bfloat16)
        m["ea_T"] = ea_T[c].astype(ml_dtypes.bfloat16)
        in_maps.append(m)
    bh2_val = float(np.asarray(inputs["bh2"]).reshape(-1)[0])
    return in_maps, K, bh2_val, (core_of, lt_of, slot)


def kernel(**inputs):
    in_maps, K, bh2_val, (core_of, lt_of, slot) = _prep(inputs)
    nc = build_nc(K, bh2_val)
    res = run_bass_kernel_spmd(
        nc, in_maps, core_ids=list(range(NCORES)), trace=TRACE
    )
    LAST_RESULT["exec_time_ns"] = res.exec_time_ns
    LAST_RESULT["res"] = res
    outs = np.stack([res.results[c]["out"] for c in range(NCORES)])  # [8, P, NT]
    return outs[core_of, slot, lt_of].astype(np.float32)


# revision 24
# speedup vs baseline: 1.0191x; 1.0191x over previous
"""Bass/Trainium2 kernel for nn_MemoryGAT (3-layer GATv2 + MLP head), 8 NeuronCores.

Nodes are degree-balanced into 8x98 tiles of 128 (K edge-tiles per node tile,
K~4). Per layer: finalize+projection loop (LN via batched Sqrt over per-tile
Sum(z)/Sum(z^2), gelu, hs/ht/skip matmuls, chunked AllGather of hs), then the
edge loop (indirect gathers of hs[src], one-hot selection matmuls, softmax
without max-subtraction, scatter-add matmul) fused with LN-stat accumulation.
ht/z stay in SBUF; activation-table churn is eliminated by keeping the ACT
engine on one function per phase (Prelu/Copy/Square live in every table).
"""

import sys
import types

sys.path.insert(0, "/opt/trn_rl_repo")

import ml_dtypes
import numpy as np
import orjson

# ---------------------------------------------------------------- shims

_counter = [0]


def _legalize_module(m, maxw=1):
    """This walrus build accepts only ONE sync-wait per instruction; hoist
    overflow waits onto NoOps inserted just before, on the same engine."""
    for f in m.get("functions", []):
        for b in f.get("blocks", []):
            insts = b.get("instructions")
            if not insts:
                continue
            out = []
            for inst in insts:
                si = inst.get("sync_info")
                waits = (si or {}).get("on_wait") or []
                if si is not None and len(waits) > maxw:
                    keep = waits[-maxw:]
                    extra = waits[: len(waits) - maxw]
                    for j in range(0, len(extra), maxw):
                        _counter[0] += 1
                        out.append(
                            {
                                "name": f"ant-wsplit-{_counter[0]}",
                                "opcode": "NoOp",
                                "engine": inst.get("engine"),
                                "ins": [],
                                "outs": [],
                                "sync_info": {
                                    "on_wait": extra[j : j + maxw],
                                    "on_update": [],
                                },
                            }
                        )
                    si["on_wait"] = keep
                out.append(inst)
            b["instructions"] = out
    return m


def _install_shims():
    import antenv

    if "antenv.axon_hooks" not in sys.modules:
        try:
            from trn_agent_boot.trn_boot import _ntff_profile_via_ctypes

            hooks = types.ModuleType("antenv.axon_hooks")
            hook = _ntff_profile_via_ctypes("/opt/axon/libaxon_pjrt.so")
            hooks.get_axon_ntff_profile_hook = lambda: hook
            hooks.set_axon_ntff_profile_hook = lambda h: None
            sys.modules["antenv.axon_hooks"] = hooks
            antenv.axon_hooks = hooks
        except Exception:
            pass

    import concourse.bass as bass
    from concourse import bass_utils

    bass_utils.upload_artifacts = lambda tmpdir: tmpdir

    if not getattr(bass.Bass, "_waitfix_installed", False):
        base = bass.Bass.to_json_bytes

        def patched(self):
            return orjson.dumps(_legalize_module(orjson.loads(base(self))))

        bass.Bass.to_json_bytes = patched
        bass.Bass._waitfix_installed = True


_install_shims()

import concourse.bass as bass
import concourse.tile as tile
from concourse import mybir
from concourse.bass_utils import run_bass_kernel_spmd

F32 = mybir.dt.float32
BF = mybir.dt.bfloat16
AF = mybir.ActivationFunctionType
ALU = mybir.AluOpType

# ---------------------------------------------------------------- sizes
N = 100_000
E = 400_000
FN = 267
DC = 256
H, D = 4, 64
HD = 256
ED = 11
NCORES = 8
P = 128
NT = 98
NPAD = NT * P  # 12544
NFULL = NCORES * NPAD
# AllGather chunk boundaries (in node tiles) and hs_full region bases
CHT = [0, 48, 80, 98]  # tile boundaries of the AG chunks
NCH = len(CHT) - 1
CHROWS = [(CHT[i + 1] - CHT[i]) * P for i in range(NCH)]  # rows/core per chunk
CHBASE = [0]
for i in range(NCH - 1):
    CHBASE.append(CHBASE[-1] + NCORES * CHROWS[i])

TRACE = False
LAST_RESULT = {}


def _row_of(lt, slot, core):
    """hs_full row for node at (core, local tile lt, slot)."""
    for c in range(NCH):
        if lt < CHT[c + 1]:
            return CHBASE[c] + core * CHROWS[c] + (lt - CHT[c]) * P + slot
    raise AssertionError


# ---------------------------------------------------------------- builder
def build_nc(K, bh2_val):
    NTK = NT * K
    ES = NTK * P

    nc = bass.Bass()
    dp = nc.declare_dram_parameter

    x_T = dp("x_T", [384, NPAD], BF, isOutput=False)
    src_c = dp("src_c", [P, NTK], mybir.dt.int32, isOutput=False)
    tgt_c = dp("tgt_c", [P, NTK], BF, isOutput=False)
    ea_T = dp("ea_T", [16, ES], BF, isOutput=False)
    wp1 = dp("wp1", [384, 64], BF, isOutput=False)
    gin_rep = dp("gin_rep", [P, 64], F32, isOutput=False)
    bin_rep = dp("bin_rep", [P, 64], F32, isOutput=False)
    iota2d = dp("iota2d", [P, P], BF, isOutput=False)
    ident = dp("ident", [P, P], BF, isOutput=False)
    wh1 = dp("wh1", [64, 32], F32, isOutput=False)
    bh1_rep = dp("bh1_rep", [P, 32], F32, isOutput=False)
    wh2_rep = dp("wh2_rep", [P, 32], F32, isOutput=False)

    LW = []
    for l, ind in ((0, 65), (1, 256), (2, 256)):
        d = {"ind": ind, "outd": 64 if l == 2 else 256}
        d["wswt"] = dp(f"wswt{l}", [ind, 512], BF, isOutput=False)
        d["we"] = dp(f"we{l}", [16, 256], BF, isOutput=False)
        d["a_rep"] = dp(f"a_rep{l}", [P, 256], BF, isOutput=False)
        if l != 1:
            d["skw"] = dp(f"skw{l}", [ind, d["outd"]], BF, isOutput=False)
            d["skb_rep"] = dp(f"skb_rep{l}", [P, d["outd"]], F32, isOutput=False)
        d["gn_rep"] = dp(f"gn_rep{l}", [P, d["outd"]], F32, isOutput=False)
        d["bn_rep"] = dp(f"bn_rep{l}", [P, d["outd"]], F32, isOutput=False)
        LW.append(d)

    out = dp("out", [P, NT], F32, isOutput=True)

    hs_shard = [nc.dram_tensor(f"hs_shard{l}", [NPAD, 256], BF) for l in range(3)]
    hs_full = [
        nc.dram_tensor(f"hs_full{l}", [NFULL, 256], BF, addr_space="Shared")
        for l in range(3)
    ]
    res0_dram = nc.dram_tensor("res0_dram", [NPAD, 256], BF)
    h1_dram = nc.dram_tensor("h1_dram", [NPAD, 256], BF)
    z_dram = nc.dram_tensor("z_dram", [NPAD, 256], BF)

    with tile.TileContext(nc) as tc:
        with (
            tc.tile_pool(name="const", bufs=1) as cpool,
            tc.tile_pool(name="work", bufs=2) as wpool,
            tc.tile_pool(name="small", bufs=2) as spool,
            tc.tile_pool(name="persist", bufs=1) as ppool,
            tc.tile_pool(name="psA", bufs=2, space="PSUM") as psA,
            tc.tile_pool(name="psB", bufs=2, space="PSUM") as psB,
            tc.tile_pool(name="psC", bufs=2, space="PSUM") as psC,
            tc.tile_pool(name="psD", bufs=2, space="PSUM") as psD,
        ):
            for v in {1e-5, 1e-8, float(bh2_val)}:
                ct = cpool.tile([P, 1], F32, tag=f"k{v}", name=f"k{_counter[0]}")
                _counter[0] += 1
                nc.vector.memset(ct[:], v)
                nc.const_aps.aps[(F32, float(v))] = ct[:]

            _cn = [0]

            def c_load(ap, shape, dt=F32):
                _cn[0] += 1
                t = cpool.tile(shape, dt, tag=f"c{_cn[0]}", name=f"c{_cn[0]}")
                nc.sync.dma_start(out=t[:], in_=ap[:])
                return t

            def c_load_chunks(ap, kk, ck, n, dt=F32):
                _cn[0] += 1
                t = cpool.tile([kk, ck * n], dt, tag=f"c{_cn[0]}", name=f"c{_cn[0]}")
                for c in range(ck):
                    nc.sync.dma_start(
                        out=t[:, c * n : (c + 1) * n],
                        in_=ap[c * kk : (c + 1) * kk, :],
                    )
                return t

            iota_sb = c_load(iota2d, [P, P], BF)
            idb_sb = c_load(ident, [P, P], BF)
            iotaK_sb = cpool.tile([P, K * P], BF, tag="iotaK", name="iotaK")
            for k in range(K):
                nc.vector.tensor_copy(iotaK_sb[:, k * P : (k + 1) * P], iota_sb[:])
            wp1_sb = c_load_chunks(wp1, P, 3, 64, BF)
            gin_sb = c_load(gin_rep, [P, 64])
            bin_sb = c_load(bin_rep, [P, 64])
            wh1_sb = c_load(wh1, [64, 32])
            bh1_sb = c_load(bh1_rep, [P, 32])
            wh2_sb = c_load(wh2_rep, [P, 32])
            lws = []
            for l, d in enumerate(LW):
                s = {}
                ck = max(d["ind"] // P, 1)
                kk = min(d["ind"], P)
                s["wswt"] = c_load_chunks(d["wswt"], kk, ck, 512, BF)
                s["we"] = c_load(d["we"], [16, 256], BF)
                s["a_rep"] = c_load(d["a_rep"], [P, 256], BF)
                if "skw" in d:
                    s["skw"] = c_load_chunks(d["skw"], kk, ck, d["outd"], BF)
                    s["skb"] = c_load(d["skb_rep"], [P, d["outd"]])
                s["gn"] = c_load(d["gn_rep"], [P, d["outd"]])
                s["bn"] = c_load(d["bn_rep"], [P, d["outd"]])
                s["ck"], s["kk"] = ck, kk
                lws.append(s)

            srcs = ppool.tile([P, NTK], mybir.dt.int32)
            nc.sync.dma_start(out=srcs[:], in_=src_c[:])
            tgts = ppool.tile([P, NTK], BF)
            nc.sync.dma_start(out=tgts[:], in_=tgt_c[:])

            h0T = ppool.tile([65, NPAD], BF)
            nc.vector.memset(h0T[64:65, :], 1.0)
            ht_all = ppool.tile([P, NT * 256], BF)
            res2_all = ppool.tile([P, NT * 64], BF)
            zview = z_dram[:].rearrange("(t p) c -> p t c", p=P)
            scores = ppool.tile([P, NT], F32)

            # one shared LN-stat set; stages are strictly phased so WAR
            # deps keep this safe
            _st = {}
            for nm in ("s1", "s2", "m", "va", "rstd"):
                _st[nm] = ppool.tile([P, NT], F32, tag=f"st{nm}", name=f"st{nm}")
            stats = [_st] * 4

            def sqrt_batch(i, dim):
                st = stats[i]
                nc.vector.tensor_scalar_mul(st["m"][:], st["s1"][:], 1.0 / dim)
                nc.vector.tensor_scalar_mul(st["va"][:], st["s2"][:], 1.0 / dim)
                nm2 = spool.tile([P, NT], F32, tag="nm2")
                nc.vector.scalar_tensor_tensor(
                    nm2[:], st["m"][:], -1.0, st["m"][:], op0=ALU.mult, op1=ALU.mult
                )
                nc.vector.tensor_add(st["va"][:], st["va"][:], nm2[:])
                sd = spool.tile([P, NT], F32, tag="sd")
                nc.scalar.activation(sd[:], st["va"][:], AF.Sqrt, bias=1e-5)
                nc.vector.reciprocal(st["rstd"][:], sd[:])
                nc.vector.scalar_tensor_tensor(
                    st["va"][:], st["m"][:], -1.0, st["rstd"][:],
                    op0=ALU.mult, op1=ALU.mult,
                )

            def ag_chunk(l, c):
                nc.gpsimd.collective_compute(
                    "AllGather",
                    ALU.bypass,
                    ins=[hs_shard[l][CHT[c] * P : CHT[c + 1] * P, :]],
                    outs=[
                        hs_full[l][CHBASE[c] : CHBASE[c] + NCORES * CHROWS[c], :]
                    ],
                    replica_groups=[list(range(NCORES))],
                )

            # ---------------- phase 0: u = x@Wp (+ctx/bias via ones-row),
            # z0 = gelu(u), accumulate LN stats -------------------------
            with nc.named_scope("p0"):
                for t in range(NT):
                    if t % 2 == 0:
                        xt = wpool.tile([P, 3, 256], BF, tag="xt", bufs=2)
                        for c in range(3):
                            nc.sync.dma_start(
                                out=xt[:, c, :],
                                in_=x_T[c * P : (c + 1) * P, t * P : (t + 2) * P],
                            )
                    xoff = (t % 2) * P
                    h0p = psD.tile([P, 64], F32, tag="proj")
                    for c in range(3):
                        nc.tensor.matmul(
                            out=h0p[:],
                            lhsT=xt[:, c, xoff : xoff + P],
                            rhs=wp1_sb[:, c * 64 : (c + 1) * 64],
                            start=(c == 0),
                            stop=(c == 2),
                        )
                    if t % 4 == 0:
                        zstag0 = wpool.tile([P, 4, 64], BF, tag="zstag0", bufs=2)
                    zslot = zstag0[:, t % 4, :]
                    nc.scalar.activation(
                        zslot, h0p[:], AF.Gelu, accum_out=stats[0]["s1"][:, t : t + 1]
                    )
                    junk = wpool.tile([P, 64], BF, tag="junk0", bufs=2)
                    nc.vector.scalar_tensor_tensor(
                        junk[:],
                        zslot,
                        1.0,
                        zslot,
                        op0=ALU.mult,
                        op1=ALU.mult,
                        accum_out=stats[0]["s2"][:, t : t + 1],
                    )
                    if t % 4 == 3 or t == NT - 1:
                        t0 = t - (t % 4)
                        nc.sync.dma_start(
                            out=zview[:, t0 : t + 1, 0:64],
                            in_=zstag0[:, : t - t0 + 1, :],
                        )
                sqrt_batch(0, 64)

            # ---------------- F2A(l): finalize h_l, project, AG ------------
            def f2a(l):
                """l in 0..3; finalize h_l from z stats[l], then projections
                for layer l (l<3) or the score head (l==3)."""
                st = stats[l]
                ind = 64 if l == 0 else (256 if l < 3 else 64)
                s = lws[l] if l < 3 else None
                for t in range(NT):
                    if t % 4 == 0:
                        nbt = min(4, NT - t)
                        z_sb = wpool.tile([P, 4, 256], BF, tag="z_sb", bufs=3)
                        nc.sync.dma_start(
                            out=z_sb[:, :nbt, :ind],
                            in_=zview[:, t : t + nbt, :ind],
                        )
                    zslot = z_sb[:, t % 4, :ind]
                    hn = wpool.tile([P, 256], BF, tag="hn", bufs=3, name="hn")[:, :ind]
                    if l == 0:
                        # g_in/b_in folded into wswt0/skw0 via the ones-row;
                        # m holds -mean*rstd after sqrt_batch
                        nc.scalar.activation(
                            hn, zslot, AF.Identity,
                            bias=st["va"][:, t : t + 1],
                            scale=st["rstd"][:, t : t + 1],
                        )
                    else:
                        t1 = wpool.tile([P, 256], F32, tag="t1", bufs=3, name="t1")[:, :ind]
                        g_sb = lws[l - 1]["gn"]
                        b_sb = lws[l - 1]["bn"]
                        nc.vector.scalar_tensor_tensor(
                            t1, zslot, st["m"][:, t : t + 1], g_sb[:, :ind],
                            op0=ALU.subtract, op1=ALU.mult,
                        )
                        u = wpool.tile([P, 256], F32, tag="u", bufs=3, name="u")[:, :ind]
                        nc.vector.scalar_tensor_tensor(
                            u, t1, st["rstd"][:, t : t + 1], b_sb[:, :ind],
                            op0=ALU.mult, op1=ALU.add,
                        )
                        nc.scalar.activation(hn, u, AF.Gelu)
                    if l == 3:
                        # score head
                        trp = psB.tile([P, P], BF, tag="tr")
                        nc.tensor.transpose(out=trp[:64, :], in_=hn, identity=idb_sb[:])
                        h3T = wpool.tile([64, P], F32, tag="h3T", bufs=2)
                        nc.scalar.copy(h3T[:], trp[:64, :])
                        sp1 = psC.tile([P, 64], F32, tag="agg", name="sp1")[:, :32]
                        nc.tensor.matmul(
                            out=sp1, lhsT=h3T[:], rhs=wh1_sb[:], start=True, stop=True
                        )
                        u1 = wpool.tile([P, 32], F32, tag="u1", bufs=2)
                        nc.vector.tensor_add(u1[:], sp1, bh1_sb[:])
                        g1 = wpool.tile([P, 32], F32, tag="g1", bufs=2)
                        nc.scalar.activation(g1[:], u1[:], AF.Gelu)
                        j32 = wpool.tile([P, 32], BF, tag="j32", bufs=2)
                        nc.vector.scalar_tensor_tensor(
                            j32[:], g1[:], 1.0, wh2_sb[:],
                            op0=ALU.mult, op1=ALU.mult,
                            accum_out=scores[:, t : t + 1],
                        )
                        continue
                    # transpose hn for projections
                    ck = s["ck"]
                    if l == 0:
                        trp = psB.tile([P, P], BF, tag="tr")
                        nc.tensor.transpose(out=trp[:64, :], in_=hn, identity=idb_sb[:])
                        nc.scalar.copy(h0T[0:64, t * P : (t + 1) * P], trp[:64, :])
                        lhs = [h0T[:, t * P : (t + 1) * P]]  # 65 rows: 64 data + ones
                    else:
                        hnT = wpool.tile([P, 2, P], BF, tag="hnT", bufs=3)
                        for c in range(2):
                            trp = psB.tile([P, P], BF, tag="tr")
                            nc.tensor.transpose(
                                out=trp[:], in_=hn[:, c * P : (c + 1) * P],
                                identity=idb_sb[:],
                            )
                            nc.scalar.copy(hnT[:, c, :], trp[:])
                        lhs = [hnT[:, c, :] for c in range(2)]
                    pp = psD if t % 2 == 0 else psA
                    hsht = pp.tile(
                        [P, 512], F32,
                        tag="proj" if t % 2 == 0 else "msg",
                        name="hsht",
                    )
                    for c in range(ck):
                        nc.tensor.matmul(
                            out=hsht[:],
                            lhsT=lhs[c],
                            rhs=s["wswt"][:, c * 512 : (c + 1) * 512],
                            start=(c == 0),
                            stop=(c == ck - 1),
                        )
                    if t % 4 == 0:
                        stag_hs = wpool.tile([P, 4, 256], BF, tag="stag_hs", bufs=2)
                    nc.scalar.copy(stag_hs[:, t % 4, :], hsht[:, 0:256])
                    nc.scalar.copy(
                        ht_all[:, t * 256 : (t + 1) * 256], hsht[:, 256:512]
                    )
                    if l != 1:
                        outd = s and LW[l]["outd"]
                        rp = psC.tile([P, 512], F32, tag="agg", name="rp")[:, :outd]
                        for c in range(ck):
                            nc.tensor.matmul(
                                out=rp,
                                lhsT=lhs[c],
                                rhs=s["skw"][:, c * outd : (c + 1) * outd],
                                start=(c == 0),
                                stop=(c == ck - 1),
                            )
                        if l == 0:
                            if t % 4 == 0:
                                stag_res = wpool.tile(
                                    [P, 4, 256], BF, tag="stag_res", bufs=2
                                )
                            nc.scalar.copy(stag_res[:, t % 4, :], rp)
                        else:
                            nc.vector.scalar_tensor_tensor(
                                res2_all[:, t * 64 : (t + 1) * 64], rp, 1.0,
                                s["skb"][:], op0=ALU.mult, op1=ALU.add,
                            )
                    if l == 1:
                        nc.sync.dma_start(
                            out=h1_dram[:].rearrange("(t p) c -> p t c", p=P)[
                                :, t, :
                            ],
                            in_=hn,
                        )
                    # batched stores + AG chunks
                    if t % 4 == 3 or t == NT - 1:
                        t0 = t - (t % 4)
                        nbt = t - t0 + 1
                        nc.sync.dma_start(
                            out=hs_shard[l][:].rearrange("(t p) c -> p t c", p=P)[
                                :, t0 : t0 + nbt, :
                            ],
                            in_=stag_hs[:, :nbt, :],
                        )
                        if l == 0:
                            nc.sync.dma_start(
                                out=res0_dram[:].rearrange("(t p) c -> p t c", p=P)[
                                    :, t0 : t0 + nbt, :
                                ],
                                in_=stag_res[:, :nbt, :],
                            )
                    for c in range(NCH):
                        if t == CHT[c + 1] - 1:
                            ag_chunk(l, c)

            # ---------------- edge + F1 loop --------------------------------
            def edge_f1(l):
                s = lws[l]
                outd = LW[l]["outd"]
                st = stats[l + 1]
                for t in range(NT):
                    if l < 2 and t % 4 == 0:
                        nbt = min(4, NT - t)
                        res_sb = wpool.tile([P, 4, 256], BF, tag="res_sb", bufs=2)
                        rdram = res0_dram if l == 0 else h1_dram
                        nc.sync.dma_start(
                            out=res_sb[:, :nbt, :],
                            in_=rdram[:].rearrange("(t p) c -> p t c", p=P)[
                                :, t : t + nbt, :
                            ],
                        )
                    if t % 2 == 0:
                        ea_sb = wpool.tile([16, 2 * K * P], BF, tag="ea_sb", bufs=2)
                        nc.sync.dma_start(
                            out=ea_sb[:],
                            in_=ea_T[:, t * K * P : (t + 2) * K * P],
                        )
                    eoff = (t % 2) * K * P
                    hs_g = wpool.tile([P, K * 256], BF, tag="hs_g", bufs=4)
                    for k in range(K):
                        nc.gpsimd.indirect_dma_start(
                            out=hs_g[:, k * 256 : (k + 1) * 256],
                            out_offset=None,
                            in_=hs_full[l][:],
                            in_offset=bass.IndirectOffsetOnAxis(
                                ap=srcs[:, t * K + k : t * K + k + 1], axis=0
                            ),
                        )
                    S_all = wpool.tile([P, K * P], BF, tag="S_all", bufs=3)
                    alph = spool.tile([P, K * 4], F32, tag="alph", bufs=2)
                    nc.vector.tensor_tensor(
                        out=S_all[:].rearrange("p (k c) -> p k c", k=K),
                        in0=iotaK_sb[:].rearrange("p (k c) -> p k c", k=K),
                        in1=tgts[:, t * K : (t + 1) * K].to_broadcast((P, K, P)),
                        op=ALU.is_equal,
                    )
                    lr_all = wpool.tile([P, K * 256], BF, tag="lr_all", bufs=4)
                    for k in range(K):
                        Sk = S_all[:, k * P : (k + 1) * P]
                        trp = psB.tile([P, P], BF, tag="tr")
                        nc.tensor.transpose(out=trp[:], in_=Sk, identity=idb_sb[:])
                        ST = wpool.tile([P, P], BF, tag="ST", bufs=3)
                        nc.scalar.copy(ST[:], trp[:])
                        msg = psA.tile([P, 256], F32, tag="msg")
                        nc.tensor.matmul(
                            out=msg[:],
                            lhsT=ea_sb[:, eoff + k * P : eoff + (k + 1) * P],
                            rhs=s["we"][:],
                            start=True,
                            stop=False,
                        )
                        nc.tensor.matmul(
                            out=msg[:], lhsT=ST[:],
                            rhs=ht_all[:, t * 256 : (t + 1) * 256],
                            start=False, stop=False,
                        )
                        nc.tensor.matmul(
                            out=msg[:], lhsT=idb_sb[:],
                            rhs=hs_g[:, k * 256 : (k + 1) * 256],
                            start=False, stop=True,
                        )
                        nc.scalar.activation(
                            lr_all[:, k * 256 : (k + 1) * 256], msg[:],
                            AF.Prelu, alpha=0.2,
                        )
                    scr_all = wpool.tile([P, K * 256], BF, tag="scr_all", bufs=2)
                    nc.vector.tensor_tensor(
                        out=scr_all[:].rearrange("p (k c) -> p k c", k=K),
                        in0=lr_all[:].rearrange("p (k c) -> p k c", k=K),
                        in1=s["a_rep"][:]
                        .rearrange("p (o c) -> p o c", o=1)
                        .to_broadcast((P, K, 256)),
                        op=ALU.mult,
                    )
                    nc.vector.tensor_reduce(
                        out=alph[:],
                        in_=scr_all[:].rearrange("p (g d) -> p g d", d=64),
                        axis=mybir.AxisListType.X,
                        op=ALU.add,
                    )
                    expa = spool.tile([P, K * 4], F32, tag="expa", bufs=2)
                    nc.scalar.activation(expa[:], alph[:], AF.Exp)
                    agg = psC.tile([P, 260], F32, tag="agg")
                    w_aug = wpool.tile([P, K * 260], BF, tag="w_aug", bufs=3)
                    w4 = w_aug[:].rearrange("p (k h c) -> p k h c", k=K, c=65)
                    nc.vector.tensor_tensor(
                        out=w4[:, :, :, 0:64],
                        in0=hs_g[:].rearrange("p (k h d) -> p k h d", k=K, d=64),
                        in1=expa[:]
                        .rearrange("p (k h) -> p k h", k=K)
                        .to_broadcast((P, K, 4, 64)),
                        op=ALU.mult,
                    )
                    nc.vector.tensor_copy(
                        w4[:, :, :, 64],
                        expa[:].rearrange("p (k h) -> p k h", k=K),
                    )
                    for k in range(K):
                        nc.tensor.matmul(
                            out=agg[:],
                            lhsT=S_all[:, k * P : (k + 1) * P],
                            rhs=w_aug[:, k * 260 : (k + 1) * 260],
                            start=(k == 0),
                            stop=(k == K - 1),
                        )
                    aggv = agg[:].rearrange("p (h c) -> p h c", c=65)
                    den = spool.tile([P, 4], F32, tag="den", bufs=2)
                    nc.vector.tensor_scalar(
                        den[:], aggv[:, :, 64], 1e-8, None, op0=ALU.add
                    )
                    rden = spool.tile([P, 4], F32, tag="rden", bufs=2)
                    nc.vector.reciprocal(rden[:], den[:])
                    gat = wpool.tile([P, 256], F32, tag="gat", bufs=2)
                    nc.vector.scalar_tensor_tensor(
                        gat[:].rearrange("p (h d) -> p h d", h=4),
                        aggv[:, :, 0:64],
                        0.25 if l == 2 else 1.0,
                        rden[:].to_broadcast((P, 4, 64)),
                        op0=ALU.mult,
                        op1=ALU.mult,
                    )
                    if l == 2:
                        g64 = wpool.tile([P, 64], F32, tag="g64", bufs=2)
                        nc.vector.tensor_reduce(
                            out=g64[:],
                            in_=gat[:].rearrange("p (h d) -> p d h", h=4),
                            axis=mybir.AxisListType.X,
                            op=ALU.add,
                        )
                        zin = g64[:]
                        res_ap = res2_all[:, t * 64 : (t + 1) * 64]
                    else:
                        zin = gat[:]
                        res_ap = res_sb[:, t % 4, :]
                    if t % 4 == 0:
                        zstag = wpool.tile([P, 4, 256], BF, tag="zstag", bufs=2)
                    zslot = zstag[:, t % 4, :outd]
                    nc.vector.scalar_tensor_tensor(
                        zslot, zin, 1.0, res_ap,
                        op0=ALU.mult, op1=ALU.add,
                        accum_out=st["s1"][:, t : t + 1],
                    )
                    junk = wpool.tile([P, 256], BF, tag="junk", bufs=2, name="junk")[:, :outd]
                    nc.scalar.activation(
                        junk, zslot, AF.Square,
                        accum_out=st["s2"][:, t : t + 1],
                    )
                    if t % 4 == 3 or t == NT - 1:
                        t0 = t - (t % 4)
                        nc.sync.dma_start(
                            out=zview[:, t0 : t + 1, :outd],
                            in_=zstag[:, : t - t0 + 1, :outd],
                        )

            with nc.named_scope("f2a0"):
                f2a(0)
            for l in range(3):
                with nc.named_scope(f"edge{l}"):
                    edge_f1(l)
                    sqrt_batch(l + 1, LW[l]["outd"])
                with nc.named_scope(f"f2a{l + 1}"):
                    f2a(l + 1)

            sig = ppool.tile([P, NT], F32)
            nc.scalar.activation(sig[:], scores[:], AF.Sigmoid, bias=bh2_val)
            nc.sync.dma_start(out=out[:], in_=sig[:])
    return nc


# ---------------------------------------------------------------- host prep
def _balance_nodes(tgt):
    """Degree-balanced assignment of nodes to NCORES*NT tiles of <=128 slots.
    Returns (gtile[node], slot[node], K)."""
    import heapq

    NTILES = NCORES * NT
    deg = np.bincount(tgt, minlength=N)
    order = np.argsort(-deg, kind="stable")
    gtile = np.empty(N, np.int32)
    slot = np.empty(N, np.int32)
    count = np.zeros(NTILES, np.int32)
    load = np.zeros(NTILES, np.int64)
    heap = [(0, t) for t in range(NTILES)]
    heapq.heapify(heap)
    for node in order:
        while True:
            ld, t = heapq.heappop(heap)
            if count[t] < P and ld == load[t]:
                break
        gtile[node] = t
        slot[node] = count[t]
        count[t] += 1
        load[t] += deg[node]
        if count[t] < P:
            heapq.heappush(heap, (int(load[t]), t))
    K = int(np.ceil(load.max() / P))
    return gtile, slot, K


def _prep(inputs):
    ei = np.asarray(inputs["edge_index"]).astype(np.int64)
    src, tgt = ei[0], ei[1]
    ea = np.asarray(inputs["edge_attr"], np.float32)

    gtile, slot, K = _balance_nodes(tgt)
    core_of = gtile // NT
    lt_of = gtile % NT

    # hs_full row id per node (chunk-major layout)
    lt = lt_of.astype(np.int64)
    chunk = np.searchsorted(np.array(CHT[1:-1]), lt, side="right")
    chrows = np.array(CHROWS)[chunk]
    chbase = np.array(CHBASE)[chunk]
    chtile0 = np.array(CHT[:-1])[chunk]
    row_id = chbase + core_of * chrows + (lt - chtile0) * P + slot

    NTK = NT * K
    ES = NTK * P

    e_core = core_of[tgt]
    e_lt = lt_of[tgt]
    e_p = slot[tgt]  # target's slot within its tile
    order = np.lexsort((e_lt, e_core))
    src_s = src[order]
    ea_s = ea[order]
    e_core_s, e_lt_s, e_p_s = e_core[order], e_lt[order], e_p[order]

    grp = e_core_s * NT + e_lt_s
    idx_in_grp = np.zeros(len(grp), np.int64)
    _, first_pos, cnt = np.unique(grp, return_index=True, return_counts=True)
    for fp, c in zip(first_pos, cnt):
        idx_in_grp[fp : fp + c] = np.arange(c)
    assert cnt.max() <= K * P, (cnt.max(), K)

    src_cols = np.zeros((NCORES, P, NTK), np.int32)
    tgt_cols = np.full((NCORES, P, NTK), -1.0, np.float32)
    ea_T = np.zeros((NCORES, 16, ES), np.float32)
    eslot = e_lt_s * (K * P) + idx_in_grp
    col = eslot // P
    row = eslot % P
    src_cols[e_core_s, row, col] = row_id[src_s].astype(np.int32)
    tgt_cols[e_core_s, row, col] = e_p_s.astype(np.float32)
    ea_T[e_core_s[:, None], np.arange(ED)[None, :], eslot[:, None]] = ea_s

    x = np.asarray(inputs["x"], np.float32)
    x_T = np.zeros((NCORES, 384, NPAD), np.float32)  # cast to bf16 per-core below
    pos = lt * P + slot  # position within core [0, NPAD)
    x_T[core_of, :FN, pos] = x
    x_T[core_of, FN, pos] = 1.0  # ones-row carries ctx@Wp+bp via wp1

    rep = lambda v: np.broadcast_to(
        np.asarray(v, np.float32)[None, :], (P, len(np.asarray(v)))
    ).copy()
    bf = lambda a: np.asarray(a).astype(ml_dtypes.bfloat16)

    Wp = np.asarray(inputs["Wp"], np.float32)
    cb = (
        np.asarray(inputs["context_vector"], np.float32) @ Wp[FN:]
        + np.asarray(inputs["bp"], np.float32)
    )
    wp1 = np.zeros((384, 64), np.float32)
    wp1[:FN] = Wp[:FN]
    wp1[FN] = cb
    wp1 = wp1.astype(ml_dtypes.bfloat16)

    common = {
        "wp1": wp1,
        "gin_rep": rep(inputs["g_in"]),
        "bin_rep": rep(inputs["b_in"]),
        "iota2d": np.broadcast_to(
            np.arange(P, dtype=np.float32)[None, :], (P, P)
        ).astype(ml_dtypes.bfloat16),
        "ident": np.eye(P, dtype=np.float32).astype(ml_dtypes.bfloat16),
        "wh1": np.asarray(inputs["Wh1"], np.float32),
        "bh1_rep": rep(inputs["bh1"]),
        "wh2_rep": rep(np.asarray(inputs["Wh2"], np.float32)[:, 0]),
    }
    g_in = np.asarray(inputs["g_in"], np.float32)
    b_in = np.asarray(inputs["b_in"], np.float32)
    for l in range(3):
        sfx = str(l)
        ws = np.asarray(inputs["Ws" + sfx], np.float32)
        wt = np.asarray(inputs["Wt" + sfx], np.float32)
        wswt = np.concatenate([ws, wt], axis=1)
        if l == 0:
            wswt = np.concatenate(
                [g_in[:, None] * wswt, (b_in @ wswt)[None, :]], axis=0
            )
        common[f"wswt{l}"] = bf(wswt)
        we = np.zeros((16, 256), np.float32)
        we[:ED] = np.asarray(inputs["We" + sfx], np.float32)
        common[f"we{l}"] = bf(we)
        common[f"a_rep{l}"] = bf(
            rep(np.asarray(inputs["A" + sfx], np.float32).reshape(-1))
        )
        if l != 1:
            skw = np.asarray(inputs[f"Sk{l}W"], np.float32)
            if l == 0:
                skw = np.concatenate(
                    [
                        g_in[:, None] * skw,
                        (b_in @ skw + np.asarray(inputs["Sk0b"], np.float32))[
                            None, :
                        ],
                    ],
                    axis=0,
                )
            common[f"skw{l}"] = bf(skw)
            common[f"skb_rep{l}"] = rep(inputs[f"Sk{l}b"])
        common[f"gn_rep{l}"] = rep(inputs["gn" + sfx])
        common[f"bn_rep{l}"] = rep(inputs["bn" + sfx])

    in_maps = []
    for c in range(NCORES):
        m = dict(common)
        m["x_T"] = x_T[c].astype(ml_dtypes.bfloat16)
        m["src_c"] = src_cols[c]
        m["tgt_c"] = tgt_cols[c].astype(ml_dtypes.bfloat16)
        m["ea_T"] = ea_T[c].astype(ml_dtypes.bfloat16)
        in_maps.append(m)
    bh2_val = float(np.asarray(inputs["bh2"]).reshape(-1)[0])
    return in_maps, K, bh2_val, (core_of, lt_of, slot)


def kernel(**inputs):
    in_maps, K, bh2_val, (core_of, lt_of, slot) = _prep(inputs)
    nc = build_nc(K, bh2_val)
    res = run_bass_kernel_spmd(
        nc, in_maps, core_ids=list(range(NCORES)), trace=TRACE
    )
    LAST_RESULT["exec_time_ns"] = res.exec_time_ns
    LAST_RESULT["res"] = res
    outs = np.stack([res.results[c]["out"] for c in range(NCORES)])  # [8, P, NT]
    return outs[core_of, slot, lt_of].astype(np.float32)
